# revision 55
# baseline (speedup 1.0000x reference)
"""DBHead (non-local attention + binarize/threshold conv branches) on 8 trn2 cores.

Sharding: 8 shards = 4 batch x 2 row-halves. Core (b, s) computes output rows
[128s, 128s+128) of the [3, 256, 256] map for batch b. All per-core variation
(which rows, halo padding, query-row masking) is pushed into host-prepared
input data so ONE SPMD program serves all 8 cores.

All matmuls run on 16-bit operands (f16 for projections/convs, bf16 for the
exp'd score tiles and V so the S@V matmul dtypes match; PSUM accumulation is
f32). 16-bit operands avoid the f32r "fp32_mode" slow path on the PE and let
DMA feed matmul operand tiles directly (no staging casts). Attention scores
are built transposed (S^T: keys on partitions) so the softmaxed tiles feed
S@V directly as the moving operand; exp runs on the scalar engine as the PSUM
eviction; the softmax denominator is accumulated on the vector engine in bf16
(2x DVE mode) and applied as a broadcast multiply. The ConvTranspose taps are
pair-packed into 128-wide outputs (dw1) and a block-diagonal lhsT (dw2) to
halve their PE cycles.
"""
import sys, os
sys.path.insert(0, "/opt/trn_rl_repo")
import numpy as np
from contextlib import ExitStack

import concourse.bass as bass
import concourse.tile as tile
from concourse import mybir, bacc
from concourse.bass_utils import run_bass_kernel_spmd

F32 = mybir.dt.float32
F16 = mybir.dt.float16
BF16 = mybir.dt.bfloat16
AFT = mybir.ActivationFunctionType
ALU = mybir.AluOpType

EPS = 1e-5
NQ = 2176  # 34 rows x 64 cols of query positions (33 real + 1 zero halo row)
QBLOCKS = [(0, 448), (448, 448), (896, 448), (1344, 448), (1792, 384)]
NKC = 32  # key chunks of 128 over 4096 positions

# wpack column offsets (all f16, rows = contraction dim on partitions)
OFF_WQ = 0        # 2 chunks x 64
OFF_WK = 128
OFF_WA = 256      # 2 chunks x 256
OFF_BZW = 768     # 18 x 64  (tap*2+chunk)
OFF_THW = 1920
OFF_DW1BZ = 3072  # 2 tap-pairs x 128 (rows 0:64)
OFF_DW1TH = 3328
OFF_DW2BZ = 3584  # block-diag [128, 8]
OFF_DW2TH = 3592
WCOLS = 3600

# bpack columns (f32)
BP_BQ, BP_BK = 0, 1
BP_BZ_S1, BP_BZ_B1, BP_BZ_S2, BP_BZ_B2 = 2, 3, 4, 5
BP_TH_S1, BP_TH_B1, BP_TH_S2, BP_TH_B2 = 6, 7, 8, 9
BP_BZ_DB2, BP_TH_DB2 = 10, 11
BCOLS = 16

_CACHE = {}
LAST_RESULTS = None


def _branch_ir(nc, tc, wr, bpk, hc, pcv, pct, ppt, pads, offw3, s1, b1,
               offdw1, s2, b2, offdw2, db2, Tt, out_d=None, dmaq=None):
    """One conv branch. If out_d is None: threshold branch, sigmoid maps are
    written into Tt and h2 scale/bias/relu runs on the (idle) Act engine.
    Else: binarize branch; h2 runs on gpsimd (Act is busy with exps then),
    prob map pieces are DMA'd to out_d[0], and binary pieces
    (sigmoid(50*(p - Tt))) to out_d[2]."""
    def emit_conv(blk):
        cv = pcv.tile([64, 512], F32, tag="cv", name="cv")
        for t in range(9):
            ky, kx = t // 3, t % 3
            for c in range(2):
                o = offw3 + (t * 2 + c) * 64
                nc.tensor.matmul(
                    cv[:], lhsT=wr[:, o:o + 64],
                    rhs=pads[c][:, blk * 8 + ky:blk * 8 + ky + 8, kx:kx + 64],
                    start=(t == 0 and c == 0), stop=(t == 8 and c == 1))
        h1t = hc.tile([64, 512], F32, tag="h1t", name="h1t")
        nc.vector.tensor_scalar(h1t[:], cv[:], bpk[0:64, s1:s1 + 1],
                                bpk[0:64, b1:b1 + 1], ALU.mult, ALU.add)
        h1c = hc.tile([64, 512], F16, tag="h1c", name="h1c")
        nc.vector.tensor_scalar_max(h1c[:], h1t[:], 0.0)
        return h1c

    bq = []  # deferred binary-map chains: run one block late so the DVE
             # prioritizes the next block's critical h1 eviction

    def flush_bq():
        for Pp, tp, lo, hi in bq:
            Dc = hc.tile([8, 512], F32, tag="Dc", name="Dc")
            nc.vector.tensor_sub(Dc[:], Pp[:], Tt[:, tp, lo:hi])
            Bp = hc.tile([8, 512], F32, tag="Bp", name="Bp")
            nc.scalar.activation(Bp[:], Dc[:], AFT.Sigmoid, scale=50.0)
            dmaq[1].dma_start(out_d[2][:, tp, lo:hi], Bp[:])
        bq.clear()

    def emit_dw(blk, h1c):
        for tp in range(2):
            ct = pct.tile([128, 512], F32, tag="ct", name="ct")
            o = offdw1 + tp * 128
            nc.tensor.matmul(ct[:], lhsT=wr[0:64, o:o + 128], rhs=h1c[:],
                             start=True, stop=True)
            h2c = hc.tile([128, 512], F16, tag="h2c", name="h2c")
            nc.scalar.activation(h2c[:], ct[:], AFT.Relu,
                                 bias=bpk[0:128, b2:b2 + 1],
                                 scale=bpk[0:128, s2:s2 + 1])
            pt = ppt.tile([8, 512], F32, tag=f"pt{tp}", name=f"pt{tp}")
            nc.tensor.matmul(pt[:], lhsT=wr[0:128, offdw2:offdw2 + 8],
                             rhs=h2c[:], start=True, stop=True)
            lo, hi = blk * 512, (blk + 1) * 512
            if out_d is None:
                nc.scalar.activation(Tt[:, tp, lo:hi], pt[:], AFT.Sigmoid,
                                     bias=bpk[0:8, db2:db2 + 1])
            else:
                Pp = hc.tile([8, 512], F32, tag="Pp", name="Pp")
                nc.scalar.activation(Pp[:], pt[:], AFT.Sigmoid,
                                     bias=bpk[0:8, db2:db2 + 1])
                dmaq[0].dma_start(out_d[0][:, tp, lo:hi], Pp[:])
                bq.append((Pp, tp, lo, hi))

    for blk in range(4):
        h1c = emit_conv(blk)
        flush_bq()  # previous block's binary chains, behind h1t/h1c on DVE
        emit_dw(blk, h1c)
    flush_bq()


def _build():
    nc = bacc.Bacc("TRN2", target_bir_lowering=False, debug=False, num_devices=8)
    xin_d = nc.dram_tensor("xin", [256, 64, 64], F16, kind="ExternalInput").ap()
    xpad_d = nc.dram_tensor("xpad", [256, 34, 66], F16, kind="ExternalInput").ap()
    qm_d = nc.dram_tensor("qmask", [1, NQ], F32, kind="ExternalInput").ap()
    wp_d = nc.dram_tensor("wpack", [128, WCOLS], F16, kind="ExternalInput").ap()
    bp_d = nc.dram_tensor("bpack", [128, BCOLS], F32, kind="ExternalInput").ap()
    ba_d = nc.dram_tensor("ba", [256], F16, kind="ExternalInput").ap()
    out_d = nc.dram_tensor("out", [3, 8, 2, 2048], F32, kind="ExternalOutput").ap()

    with tile.TileContext(nc) as tc, ExitStack() as ctx:
        cp = ctx.enter_context(tc.tile_pool(name="const", bufs=1))
        pp = ctx.enter_context(tc.tile_pool(name="pads", bufs=1))

        # ---- constants + inputs: direct 16-bit DMA, no staging. Queue
        # assignment is ordered so the threshold branch (wr th-cols + xpad)
        # can start ~2us in ----
        bpk = cp.tile([128, BCOLS], F32)
        nc.sync.dma_start(bpk[:], bp_d[:])
        wr = cp.tile([128, WCOLS], F16)
        nc.sync.dma_start(wr[:, OFF_WQ:OFF_WQ + 128], wp_d[:, OFF_WQ:OFF_WQ + 128])
        nc.sync.dma_start(wr[:, OFF_THW:], wp_d[:, OFF_THW:])
        qm = cp.tile([1, NQ], F32)
        bar2 = cp.tile([1, 2, 256], F16)
        onesc = cp.tile([128, 1], BF16)
        nc.vector.memset(onesc[:], 1.0)
        onesrB = cp.tile([1, 128], BF16)
        nc.vector.memset(onesrB[:], 1.0)
        onesrV = cp.tile([1, 128], F16)
        nc.vector.memset(onesrV[:], 1.0)
        zc = cp.tile([128, 34], F16)
        nc.gpsimd.memset(zc[:], 0.0)
        Tt = cp.tile([8, 2, 2048], F32)  # threshold sigmoid maps

        xpr = [pp.tile([128, 34, 66], F16, tag=f"xp{c}", name=f"xpr{c}")
               for c in range(2)]
        xnp = [pp.tile([128, 34, 66], F16, tag=f"xn{c}", name=f"xnp{c}")
               for c in range(2)]
        # blk0's rows (both chunks) first; the bigger second piece is split
        # across the two HWDGE queues so it streams in parallel
        for c in range(2):
            nc.scalar.dma_start(
                xpr[c][:, 0:10].rearrange("p r c2 -> p (r c2)"),
                xpad_d[c * 128:(c + 1) * 128, 0:10].rearrange(
                    "p r c2 -> p (r c2)"))
        for c, eng in ((0, nc.sync), (1, nc.scalar)):
            eng.dma_start(
                xpr[c][:, 10:34].rearrange("p r c2 -> p (r c2)"),
                xpad_d[c * 128:(c + 1) * 128, 10:34].rearrange(
                    "p r c2 -> p (r c2)"))
        # zero borders of xn pads (cols 0/65); rows are fully written later
        for c in range(2):
            for col in (0, 65):
                nc.gpsimd.tensor_copy(
                    xnp[c][:, :, col:col + 1],
                    zc[:].rearrange("p (r o) -> p r o", o=1))

        with tc.tile_pool(name="att", bufs=1) as ap_, \
             tc.tile_pool(name="eb", bufs=4) as eb, \
             tc.tile_pool(name="rc", bufs=2) as rc, \
             ExitStack() as a2:
            e1r = ap_.tile([64, NQ], F16)
            e2r = ap_.tile([64, 4096], F16)
            V = ap_.tile([128, NKC, 256], BF16)
            NP = NKC // 2
            pre = {}  # (qi, jj) -> E tile, emitted early during the V loop
            psc = None  # PSUM pool, opened once the th-branch pools close

            def emit_qk(qi, jj):
                q0, w = QBLOCKS[qi]
                sc = psc.tile([128, 2, 512], F32, tag="sc", name="sc")
                for u in range(2):
                    j = 2 * jj + u
                    nc.tensor.matmul(sc[:, u, :w],
                                     lhsT=e2r[:, j * 128:(j + 1) * 128],
                                     rhs=e1r[:, q0:q0 + w],
                                     start=True, stop=True)
                E = eb.tile([128, 2, 512], BF16, tag="E", name="E")
                nc.scalar.activation(E[:, :, :w], sc[:, :, :w], AFT.Exp)
                return E

            with tc.tile_pool(name="xr", bufs=1) as xp:
                xr = [xp.tile([128, 64, 64], F16, tag=f"xr{c}", name=f"xr{c}")
                      for c in range(2)]

                # e1 first: its gate is tiny (128 weight cols + first xpad
                # rows), so the PE starts ~2us earlier and warms HAM sooner
                pe_ctx = ExitStack()
                pe = pe_ctx.enter_context(
                    tc.tile_pool(name="pe", bufs=2, space="PSUM"))
                for q0, w in QBLOCKS:
                    r0, nr = q0 // 64, w // 64
                    p = pe.tile([64, 512], F32, tag="pe", name="pe_t")
                    for c in range(2):
                        o = OFF_WQ + c * 64
                        nc.tensor.matmul(p[:, :w], lhsT=wr[:, o:o + 64],
                                         rhs=xpr[c][:, r0:r0 + nr, 1:65],
                                         start=(c == 0), stop=(c == 1))
                    nc.scalar.activation(e1r[:, q0:q0 + w], p[:, :w], AFT.Prelu,
                                         bias=bpk[0:64, BP_BQ:BP_BQ + 1],
                                         alpha=0.25)

                # threshold branch: needs only wr th-cols + xpr + bpk; fills
                # the PE while the remaining inputs stream in
                with tc.tile_pool(name="hct", bufs=3) as hct, \
                     tc.tile_pool(name="pcv0", bufs=2, space="PSUM") as pcv0, \
                     tc.tile_pool(name="pct0", bufs=2, space="PSUM") as pct0, \
                     tc.tile_pool(name="ppt0", bufs=1, space="PSUM") as ppt0:
                    _branch_ir(nc, tc, wr, bpk, hct, pcv0, pct0, ppt0, xpr,
                               OFF_THW, BP_TH_S1, BP_TH_B1, OFF_DW1TH,
                               BP_TH_S2, BP_TH_B2, OFF_DW2TH, BP_TH_DB2, Tt)
                # deferred loads (not needed by the threshold branch): emitted
                # after it so the DMA engine prioritizes the th gate. All on
                # the sync HWDGE queue, ahead of the (blocking) Tt store.
                nc.sync.dma_start(wr[:, :OFF_THW], wp_d[:, :OFF_THW])
                for c, eng in ((0, nc.sync), (1, nc.scalar)):
                    eng.dma_start(
                        xr[c][:].rearrange("p r c2 -> p (r c2)"),
                        xin_d[c * 128:(c + 1) * 128].rearrange("p r c2 -> p (r c2)"))
                nc.sync.dma_start(qm[:], qm_d[:])
                ba_row = bass.AP(tensor=ba_d.tensor, offset=ba_d.offset,
                                 ap=[[0, 1]] + [list(a) for a in ba_d.ap])
                nc.scalar.dma_start(bar2[:, 0, :], ba_row)
                nc.scalar.dma_start(bar2[:, 1, :], ba_row)
                nc.scalar.dma_start(
                    out_d[1].rearrange("p u c2 -> p (u c2)"),
                    Tt[:].rearrange("p u c2 -> p (u c2)"))

                xr_f = [t[:].rearrange("p r c2 -> p (r c2)") for t in xr]

                # ---- phase 1: e2 (keys), V (values, pos-major) ----
                pe_ctx.close()  # free e1's banks (PSUM pools pop LIFO)
                psc = a2.enter_context(
                    tc.tile_pool(name="psc", bufs=2, space="PSUM"))
                with tc.tile_pool(name="pe2", bufs=2, space="PSUM") as pe2, \
                     tc.tile_pool(name="pv", bufs=2, space="PSUM") as pv:
                    for k0 in range(0, 4096, 512):
                        p = pe2.tile([64, 512], F32, tag="pe", name="pe_t")
                        for c in range(2):
                            o = OFF_WK + c * 64
                            nc.tensor.matmul(p[:], lhsT=wr[:, o:o + 64],
                                             rhs=xr_f[c][:, k0:k0 + 512],
                                             start=(c == 0), stop=(c == 1))
                        nc.scalar.activation(e2r[:, k0:k0 + 512], p[:], AFT.Prelu,
                                             bias=bpk[0:64, BP_BK:BP_BK + 1],
                                             alpha=0.25)
                    for jj in range(NKC // 2):
                        p = pv.tile([128, 2, 256], F32, tag="pv", name="pv_t")
                        for u in range(2):
                            j = 2 * jj + u
                            for c in range(2):
                                o = OFF_WA + c * 256
                                nc.tensor.matmul(p[:, u, :],
                                                 lhsT=xr_f[c][:, j * 128:(j + 1) * 128],
                                                 rhs=wr[:, o:o + 256],
                                                 start=(c == 0), stop=False)
                            nc.tensor.matmul(p[:, u, :], lhsT=onesrV[:],
                                             rhs=bar2[:, u, :], start=False,
                                             stop=True)
                        if jj % 2 == 0:
                            nc.scalar.activation(V[:, 2 * jj:2 * jj + 2, :],
                                                 p[:], AFT.Prelu, alpha=0.25)
                        else:
                            # alternate evictions onto the DVE so the V loop
                            # is not gated by the Act engine (which also runs
                            # the interleaved warm-up exps here)
                            vt = rc.tile([128, 2, 256], F16, tag="vt", name="vt")
                            nc.vector.tensor_scalar_mul(vt[:], p[:], 0.25)
                            nc.vector.tensor_max(V[:, 2 * jj:2 * jj + 2, :],
                                                 vt[:], p[:])
                        # warm the score pipeline: first q-block's QK/exp
                        # pairs interleave with V production, so phase 2
                        # starts with exp'd tiles banked and Act drained
                        if jj in (4, 9, 14):
                            pre[(0, (jj - 4) // 5)] = emit_qk(0, (jj - 4) // 5)

            # ---- phase 2: attention blocks, software-pipelined by one key
            # pair so SV matmuls never wait on the exp eviction; the q-block
            # tail (denominator + normalize) is deferred one extra pair so
            # the PE never waits on the DVE accumulation chain ----
            with tc.tile_pool(name="pxn", bufs=1, space="PSUM") as pxn, \
                 tc.tile_pool(name="prs", bufs=1, space="PSUM") as prs, \
                 tc.tile_pool(name="prb", bufs=1, space="PSUM") as prb:
                state = {}  # per-qblock: xn_ps tiles, racc accumulators

                def get_state(qi):
                    if qi not in state:
                        state[qi] = {
                            "xn": [pxn.tile([128, 512], F32, tag=f"xnp{t}",
                                            name=f"xnps{t}") for t in range(2)],
                            "racc": [rc.tile([128, 512], BF16, tag="racc",
                                             name="racc"),
                                     rc.tile([128, 512], BF16, tag="rac2",
                                             name="rac2")],
                        }
                    return state[qi]

                def emit_sv(qi, jj, E):
                    q0, w = QBLOCKS[qi]
                    st = get_state(qi)
                    for u in range(2):
                        j = 2 * jj + u
                        for t in range(2):
                            nc.tensor.matmul(
                                st["xn"][t][:, :w],
                                lhsT=V[:, j, t * 128:(t + 1) * 128],
                                rhs=E[:, u, :w],
                                start=(jj == 0 and u == 0),
                                stop=(jj == NP - 1 and u == 1))

                def emit_racc(qi, jj, E):
                    q0, w = QBLOCKS[qi]
                    st = get_state(qi)
                    ps = rc.tile([128, 512], BF16, tag="ps", name="ps")
                    nc.vector.tensor_add(ps[:, :w], E[:, 0, :w], E[:, 1, :w])
                    acc = st["racc"][jj % 2]
                    if jj < 2:
                        nc.vector.tensor_copy(acc[:, :w], ps[:, :w])
                    else:
                        nc.vector.tensor_add(acc[:, :w], acc[:, :w], ps[:, :w])

                def emit_tail(qi, Elast):
                    q0, w = QBLOCKS[qi]
                    r0, nr = q0 // 64, w // 64
                    st = state.pop(qi)
                    racc, rac2 = st["racc"]
                    rs = prs.tile([1, 512], F32, tag="rs", name="rs")
                    rb = prb.tile([128, 512], F32, tag="rb", name="rb")
                    # denominator accumulated on the PE across both bf16
                    # accumulators (no DVE combine on the critical path)
                    nc.tensor.matmul(rs[:, :w], lhsT=onesc[:], rhs=racc[:, :w],
                                     start=True, stop=False)
                    nc.tensor.matmul(rs[:, :w], lhsT=onesc[:], rhs=rac2[:, :w],
                                     start=False, stop=True)
                    rrow = rc.tile([1, 512], F32, tag="rrow", name="rrow")
                    nc.vector.reciprocal_approx_fast(rrow[:, :w], rs[:, :w])
                    rrm = rc.tile([1, 512], BF16, tag="rrm", name="rrm")
                    nc.vector.tensor_mul(rrm[:, :w], rrow[:, :w], qm[:, q0:q0 + w])
                    # keep-warm fillers: dep-free matmuls into the rb bank so
                    # the PE never idles (and HAM never re-throttles) while
                    # the reciprocal chain runs on the DVE
                    for _ in range(3):
                        nc.tensor.matmul(rb[:, :w], lhsT=V[:, 0, 0:128],
                                         rhs=Elast[:, 1, :w], start=True, stop=True)
                    nc.tensor.matmul(rb[:, :w], lhsT=onesrB[:], rhs=rrm[:, :w],
                                     start=True, stop=True)
                    rbs = rc.tile([128, 512], F32, tag="rbs", name="rbs")
                    nc.vector.tensor_copy(rbs[:, :w], rb[:, :w])
                    # more fillers into the (now idle) rs bank while the DVE
                    # evicts and normalizes xn
                    for _ in range(4):
                        nc.tensor.matmul(rs[:, :w], lhsT=onesc[:],
                                         rhs=Elast[:, 0, :w], start=True, stop=True)
                    for t in range(2):
                        nc.vector.tensor_mul(
                            xnp[t][:, r0:r0 + nr, 1:65],
                            st["xn"][t][:, :w].rearrange("p (r c2) -> p r c2", c2=64),
                            rbs[:, :w].rearrange("p (r c2) -> p r c2", c2=64))

                # drain the pre-emitted pairs, then run the pipeline
                for jj in (0, 1):
                    emit_sv(0, jj, pre[(0, jj)])
                    emit_racc(0, jj, pre[(0, jj)])
                prev = (0, 2, pre[(0, 2)])
                pending = None
                items = [(0, jj) for jj in range(3, NP)] + \
                        [(qi, jj) for qi in range(1, len(QBLOCKS))
                         for jj in range(NP)]
                for qi, jj in items:
                    E = emit_qk(qi, jj)
                    if pending is not None and jj == 1:
                        emit_tail(*pending)
                        pending = None
                    pqi, pjj, pE = prev
                    emit_sv(pqi, pjj, pE)
                    emit_racc(pqi, pjj, pE)
                    if pjj == NP - 1:
                        pending = (pqi, pE)
                    prev = (qi, jj, E)
                pqi, pjj, pE = prev
                emit_sv(pqi, pjj, pE)
                emit_racc(pqi, pjj, pE)
                emit_tail(pqi, pE)

        # ---- phase 3: binarize branch (att pool closed; SBUF freed) ----
        with tc.tile_pool(name="hc", bufs=3) as hc, \
             tc.tile_pool(name="pcv", bufs=2, space="PSUM") as pcv, \
             tc.tile_pool(name="pct", bufs=2, space="PSUM") as pct, \
             tc.tile_pool(name="ppt", bufs=2, space="PSUM") as ppt:
            _branch_ir(nc, tc, wr, bpk, hc, pcv, pct, ppt, xnp, OFF_BZW,
                       BP_BZ_S1, BP_BZ_B1, OFF_DW1BZ, BP_BZ_S2, BP_BZ_B2,
                       OFF_DW2BZ, BP_BZ_DB2, Tt, out_d=out_d,
                       dmaq=[nc.scalar, nc.sync])

    nc.compile()
    return nc


def _prep(inputs):
    """Host-side parameter prep shared by all cores (numpy, tiny)."""
    g = {k: np.asarray(v, np.float32) for k, v in inputs.items()}
    wpack = np.zeros((128, WCOLS), np.float32)
    wqT = g["wm1"].reshape(64, 256).T
    wpack[:, OFF_WQ:OFF_WQ + 64] = wqT[0:128]
    wpack[:, OFF_WQ + 64:OFF_WQ + 128] = wqT[128:256]
    wkT = g["wm2"].reshape(64, 256).T
    wpack[:, OFF_WK:OFF_WK + 64] = wkT[0:128]
    wpack[:, OFF_WK + 64:OFF_WK + 128] = wkT[128:256]
    waT = g["wa"].reshape(256, 256).T
    wpack[:, OFF_WA:OFF_WA + 256] = waT[0:128]
    wpack[:, OFF_WA + 256:OFF_WA + 512] = waT[128:256]
    for name, off in (("bz_cw", OFF_BZW), ("th_cw", OFF_THW)):
        w3 = g[name].transpose(2, 3, 1, 0).reshape(9, 256, 64)
        for t in range(9):
            for c in range(2):
                wpack[:, off + (t * 2 + c) * 64:off + (t * 2 + c) * 64 + 64] = \
                    w3[t, c * 128:(c + 1) * 128]
    # conv_transpose flips the kernel: tap (di,dj) uses w[1-di, 1-dj];
    # dw1 taps pair-packed into 128-wide outputs
    for name, off in (("bz_dw1", OFF_DW1BZ), ("th_dw1", OFF_DW1TH)):
        d1 = g[name].reshape(4, 64, 64)[::-1]
        for tp in range(2):
            wpack[0:64, off + tp * 128:off + tp * 128 + 64] = d1[2 * tp]
            wpack[0:64, off + tp * 128 + 64:off + tp * 128 + 128] = d1[2 * tp + 1]
    # dw2 block-diagonal [128, 8]: both pair members share the same 64->4 map
    for name, off in (("bz_dw2", OFF_DW2BZ), ("th_dw2", OFF_DW2TH)):
        w2 = g[name].transpose(2, 0, 1, 3).reshape(64, 4)[:, ::-1]
        wpack[0:64, off:off + 4] = w2
        wpack[64:128, off + 4:off + 8] = w2

    bpack = np.zeros((128, BCOLS), np.float32)
    bpack[0:64, BP_BQ] = g["bm1"]
    bpack[0:64, BP_BK] = g["bm2"]
    for pre, (cs1, cb1, cs2, cb2, cdb2) in (
            ("bz", (BP_BZ_S1, BP_BZ_B1, BP_BZ_S2, BP_BZ_B2, BP_BZ_DB2)),
            ("th", (BP_TH_S1, BP_TH_B1, BP_TH_S2, BP_TH_B2, BP_TH_DB2))):
        inv1 = g[f"{pre}_g1"] / np.sqrt(g[f"{pre}_v1"] + EPS)
        bpack[0:64, cs1] = inv1
        bpack[0:64, cb1] = g[f"{pre}_b1"] - g[f"{pre}_m1"] * inv1
        inv2 = g[f"{pre}_g2"] / np.sqrt(g[f"{pre}_v2"] + EPS)
        s2v = np.concatenate([inv2, inv2])
        b2v = np.concatenate([g[f"{pre}_b2"] + (g[f"{pre}_db1"] - g[f"{pre}_m2"]) * inv2] * 2)
        bpack[:, cs2] = s2v
        bpack[:, cb2] = b2v
        bpack[0:8, cdb2] = float(g[f"{pre}_db2"][0])
    return g, wpack, bpack


def kernel(**inputs):
    global LAST_RESULTS
    if "nc" not in _CACHE:
        _CACHE["nc"] = _build()
    nc = _CACHE["nc"]
    g, wpack, bpack = _prep(inputs)
    x = g["x"]  # [4, 256, 64, 64]
    wpack16 = wpack.astype(np.float16)
    ba16 = g["ba"].astype(np.float16)

    in_maps = []
    for core in range(8):
        b, s = core % 4, core // 4
        xpad = np.zeros((256, 34, 66), np.float32)
        qmask = np.ones((1, NQ), np.float32)
        if s == 0:
            xpad[:, 1:34, 1:65] = x[b][:, 0:33]
            qmask[0, 0:64] = 0.0
        else:
            xpad[:, 0:33, 1:65] = x[b][:, 31:64]
            qmask[0, 33 * 64:] = 0.0
        in_maps.append({"xin": np.ascontiguousarray(x[b]).astype(np.float16),
                        "xpad": xpad.astype(np.float16),
                        "qmask": qmask, "wpack": wpack16, "bpack": bpack,
                        "ba": ba16})

    br = run_bass_kernel_spmd(
        nc, in_maps, core_ids=list(range(8)),
        trace=os.environ.get("KERNEL_TRACE", "0") == "1")
    LAST_RESULTS = br

    out = np.zeros((4, 3, 256, 256), np.float32)
    for core in range(8):
        b, s = core % 4, core // 4
        # raw [ch, m*4 + (di2*2+dj2), tp, r*64+c]:
        # final row = 4r + 2*tp + di2, col = 4c + 2*m + dj2
        raw = br.results[core]["out"].reshape(3, 2, 2, 2, 2, 32, 64)
        half = raw.transpose(0, 5, 4, 2, 6, 1, 3).reshape(3, 128, 256)
        out[b, :, 128 * s:128 * (s + 1), :] = half
    return out


# revision 58
# speedup vs baseline: 1.0095x; 1.0095x over previous
"""DBHead (non-local attention + binarize/threshold conv branches) on 8 trn2 cores.

Sharding: 8 shards = 4 batch x 2 row-halves. Core (b, s) computes output rows
[128s, 128s+128) of the [3, 256, 256] map for batch b. All per-core variation
(which rows, halo padding, query-row masking) is pushed into host-prepared
input data so ONE SPMD program serves all 8 cores.

All matmuls run on 16-bit operands (f16 for projections/convs, bf16 for the
exp'd score tiles and V so the S@V matmul dtypes match; PSUM accumulation is
f32). 16-bit operands avoid the f32r "fp32_mode" slow path on the PE and let
DMA feed matmul operand tiles directly (no staging casts). Attention scores
are built transposed (S^T: keys on partitions) so the softmaxed tiles feed
S@V directly as the moving operand; exp runs on the scalar engine as the PSUM
eviction; the softmax denominator is accumulated on the vector engine in bf16
(2x DVE mode) and applied as a broadcast multiply. The ConvTranspose taps are
pair-packed into 128-wide outputs (dw1) and a block-diagonal lhsT (dw2) to
halve their PE cycles.
"""
import sys, os
sys.path.insert(0, "/opt/trn_rl_repo")
import numpy as np
from contextlib import ExitStack

import concourse.bass as bass
import concourse.tile as tile
from concourse import mybir, bacc
from concourse.bass_utils import run_bass_kernel_spmd

F32 = mybir.dt.float32
F16 = mybir.dt.float16
BF16 = mybir.dt.bfloat16
AFT = mybir.ActivationFunctionType
ALU = mybir.AluOpType

EPS = 1e-5
NQ = 2176  # 34 rows x 64 cols of query positions (33 real + 1 zero halo row)
QBLOCKS = [(0, 448), (448, 448), (896, 448), (1344, 448), (1792, 384)]
NKC = 32  # key chunks of 128 over 4096 positions

# wpack column offsets (all f16, rows = contraction dim on partitions)
OFF_WQ = 0        # 2 chunks x 64
OFF_WK = 128
OFF_WA = 256      # 2 chunks x 256
OFF_BZW = 768     # 18 x 64  (tap*2+chunk)
OFF_THW = 1920
OFF_DW1BZ = 3072  # 2 tap-pairs x 128 (rows 0:64)
OFF_DW1TH = 3328
OFF_DW2BZ = 3584  # block-diag [128, 8]
OFF_DW2TH = 3592
WCOLS = 3600

# bpack columns (f32)
BP_BQ, BP_BK = 0, 1
BP_BZ_S1, BP_BZ_B1, BP_BZ_S2, BP_BZ_B2 = 2, 3, 4, 5
BP_TH_S1, BP_TH_B1, BP_TH_S2, BP_TH_B2 = 6, 7, 8, 9
BP_BZ_DB2, BP_TH_DB2 = 10, 11
BCOLS = 16

_CACHE = {}
LAST_RESULTS = None


def _branch_ir(nc, tc, wr, bpk, hc, pcv, pct, ppt, pads, offw3, s1, b1,
               offdw1, s2, b2, offdw2, db2, Tt, out_d=None, dmaq=None):
    """One conv branch. If out_d is None: threshold branch, sigmoid maps are
    written into Tt and h2 scale/bias/relu runs on the (idle) Act engine.
    Else: binarize branch; h2 runs on gpsimd (Act is busy with exps then),
    prob map pieces are DMA'd to out_d[0], and binary pieces
    (sigmoid(50*(p - Tt))) to out_d[2]."""
    def emit_conv(blk):
        cv = pcv.tile([64, 512], F32, tag="cv", name="cv")
        for t in range(9):
            ky, kx = t // 3, t % 3
            for c in range(2):
                o = offw3 + (t * 2 + c) * 64
                nc.tensor.matmul(
                    cv[:], lhsT=wr[:, o:o + 64],
                    rhs=pads[c][:, blk * 8 + ky:blk * 8 + ky + 8, kx:kx + 64],
                    start=(t == 0 and c == 0), stop=(t == 8 and c == 1))
        h1t = hc.tile([64, 512], F32, tag="h1t", name="h1t")
        nc.vector.tensor_scalar(h1t[:], cv[:], bpk[0:64, s1:s1 + 1],
                                bpk[0:64, b1:b1 + 1], ALU.mult, ALU.add)
        h1c = hc.tile([64, 512], F16, tag="h1c", name="h1c")
        nc.vector.tensor_scalar_max(h1c[:], h1t[:], 0.0)
        return h1c

    def emit_dw(blk, h1c):
        for tp in range(2):
            ct = pct.tile([128, 512], F32, tag="ct", name="ct")
            o = offdw1 + tp * 128
            nc.tensor.matmul(ct[:], lhsT=wr[0:64, o:o + 128], rhs=h1c[:],
                             start=True, stop=True)
            h2c = hc.tile([128, 512], F16, tag="h2c", name="h2c")
            nc.scalar.activation(h2c[:], ct[:], AFT.Relu,
                                 bias=bpk[0:128, b2:b2 + 1],
                                 scale=bpk[0:128, s2:s2 + 1])
            pt = ppt.tile([8, 512], F32, tag=f"pt{tp}", name=f"pt{tp}")
            nc.tensor.matmul(pt[:], lhsT=wr[0:128, offdw2:offdw2 + 8],
                             rhs=h2c[:], start=True, stop=True)
            lo, hi = blk * 512, (blk + 1) * 512
            if out_d is None:
                nc.scalar.activation(Tt[:, tp, lo:hi], pt[:], AFT.Sigmoid,
                                     bias=bpk[0:8, db2:db2 + 1])
            else:
                Pp = hc.tile([8, 512], F32, tag="Pp", name="Pp")
                nc.scalar.activation(Pp[:], pt[:], AFT.Sigmoid,
                                     bias=bpk[0:8, db2:db2 + 1])
                dmaq[0].dma_start(out_d[0][:, tp, lo:hi], Pp[:])
                Dc = hc.tile([8, 512], F32, tag="Dc", name="Dc")
                nc.vector.tensor_sub(Dc[:], Pp[:], Tt[:, tp, lo:hi])
                Bp = hc.tile([8, 512], F32, tag="Bp", name="Bp")
                nc.scalar.activation(Bp[:], Dc[:], AFT.Sigmoid, scale=50.0)
                dmaq[1].dma_start(out_d[2][:, tp, lo:hi], Bp[:])

    for blk in range(4):
        emit_dw(blk, emit_conv(blk))


def _build():
    nc = bacc.Bacc("TRN2", target_bir_lowering=False, debug=False, num_devices=8)
    xin_d = nc.dram_tensor("xin", [256, 64, 64], F16, kind="ExternalInput").ap()
    xpad_d = nc.dram_tensor("xpad", [256, 34, 66], F16, kind="ExternalInput").ap()
    qm_d = nc.dram_tensor("qmask", [1, NQ], F32, kind="ExternalInput").ap()
    wp_d = nc.dram_tensor("wpack", [128, WCOLS], F16, kind="ExternalInput").ap()
    bp_d = nc.dram_tensor("bpack", [128, BCOLS], F32, kind="ExternalInput").ap()
    ba_d = nc.dram_tensor("ba", [256], F16, kind="ExternalInput").ap()
    out_d = nc.dram_tensor("out", [3, 8, 2, 2048], F32, kind="ExternalOutput").ap()

    with tile.TileContext(nc) as tc, ExitStack() as ctx:
        cp = ctx.enter_context(tc.tile_pool(name="const", bufs=1))
        pp = ctx.enter_context(tc.tile_pool(name="pads", bufs=1))

        # ---- constants + inputs: direct 16-bit DMA, no staging. Queue
        # assignment is ordered so the threshold branch (wr th-cols + xpad)
        # can start ~2us in ----
        bpk = cp.tile([128, BCOLS], F32)
        nc.sync.dma_start(bpk[:], bp_d[:])
        wr = cp.tile([128, WCOLS], F16)
        nc.sync.dma_start(wr[:, OFF_WQ:OFF_WQ + 128], wp_d[:, OFF_WQ:OFF_WQ + 128])
        nc.sync.dma_start(wr[:, OFF_THW:], wp_d[:, OFF_THW:])
        qm = cp.tile([1, NQ], F32)
        bar2 = cp.tile([1, 2, 256], F16)
        onesc = cp.tile([128, 1], BF16)
        nc.vector.memset(onesc[:], 1.0)
        onesrB = cp.tile([1, 128], BF16)
        nc.vector.memset(onesrB[:], 1.0)
        onesrV = cp.tile([1, 128], F16)
        nc.vector.memset(onesrV[:], 1.0)
        zc = cp.tile([128, 34], F16)
        nc.gpsimd.memset(zc[:], 0.0)
        Tt = cp.tile([8, 2, 2048], F32)  # threshold sigmoid maps

        xpr = [pp.tile([128, 34, 66], F16, tag=f"xp{c}", name=f"xpr{c}")
               for c in range(2)]
        xnp = [pp.tile([128, 34, 66], F16, tag=f"xn{c}", name=f"xnp{c}")
               for c in range(2)]
        # blk0's rows (both chunks) first; the bigger second piece is split
        # across the two HWDGE queues so it streams in parallel
        for c in range(2):
            nc.scalar.dma_start(
                xpr[c][:, 0:10].rearrange("p r c2 -> p (r c2)"),
                xpad_d[c * 128:(c + 1) * 128, 0:10].rearrange(
                    "p r c2 -> p (r c2)"))
        for c, eng in ((0, nc.sync), (1, nc.scalar)):
            eng.dma_start(
                xpr[c][:, 10:34].rearrange("p r c2 -> p (r c2)"),
                xpad_d[c * 128:(c + 1) * 128, 10:34].rearrange(
                    "p r c2 -> p (r c2)"))
        # zero borders of xn pads (cols 0/65); rows are fully written later
        for c in range(2):
            for col in (0, 65):
                nc.gpsimd.tensor_copy(
                    xnp[c][:, :, col:col + 1],
                    zc[:].rearrange("p (r o) -> p r o", o=1))

        with tc.tile_pool(name="att", bufs=1) as ap_, \
             tc.tile_pool(name="eb", bufs=4) as eb, \
             tc.tile_pool(name="rc", bufs=2) as rc, \
             ExitStack() as a2:
            e1r = ap_.tile([64, NQ], F16)
            e2r = ap_.tile([64, 4096], F16)
            V = ap_.tile([128, NKC, 256], BF16)
            NP = NKC // 2
            pre = {}  # (qi, jj) -> E tile, emitted early during the V loop
            psc = None  # PSUM pool, opened once the th-branch pools close

            def emit_qk(qi, jj):
                q0, w = QBLOCKS[qi]
                sc = psc.tile([128, 2, 512], F32, tag="sc", name="sc")
                for u in range(2):
                    j = 2 * jj + u
                    nc.tensor.matmul(sc[:, u, :w],
                                     lhsT=e2r[:, j * 128:(j + 1) * 128],
                                     rhs=e1r[:, q0:q0 + w],
                                     start=True, stop=True)
                E = eb.tile([128, 2, 512], BF16, tag="E", name="E")
                nc.scalar.activation(E[:, :, :w], sc[:, :, :w], AFT.Exp)
                return E

            with tc.tile_pool(name="xr", bufs=1) as xp:
                xr = [xp.tile([128, 64, 64], F16, tag=f"xr{c}", name=f"xr{c}")
                      for c in range(2)]

                # e1 first: its gate is tiny (128 weight cols + first xpad
                # rows), so the PE starts ~2us earlier and warms HAM sooner
                pe_ctx = ExitStack()
                pe = pe_ctx.enter_context(
                    tc.tile_pool(name="pe", bufs=2, space="PSUM"))
                for q0, w in QBLOCKS:
                    r0, nr = q0 // 64, w // 64
                    p = pe.tile([64, 512], F32, tag="pe", name="pe_t")
                    for c in range(2):
                        o = OFF_WQ + c * 64
                        nc.tensor.matmul(p[:, :w], lhsT=wr[:, o:o + 64],
                                         rhs=xpr[c][:, r0:r0 + nr, 1:65],
                                         start=(c == 0), stop=(c == 1))
                    nc.scalar.activation(e1r[:, q0:q0 + w], p[:, :w], AFT.Prelu,
                                         bias=bpk[0:64, BP_BQ:BP_BQ + 1],
                                         alpha=0.25)

                # threshold branch: needs only wr th-cols + xpr + bpk; fills
                # the PE while the remaining inputs stream in
                with tc.tile_pool(name="hct", bufs=3) as hct, \
                     tc.tile_pool(name="pcv0", bufs=2, space="PSUM") as pcv0, \
                     tc.tile_pool(name="pct0", bufs=2, space="PSUM") as pct0, \
                     tc.tile_pool(name="ppt0", bufs=1, space="PSUM") as ppt0:
                    _branch_ir(nc, tc, wr, bpk, hct, pcv0, pct0, ppt0, xpr,
                               OFF_THW, BP_TH_S1, BP_TH_B1, OFF_DW1TH,
                               BP_TH_S2, BP_TH_B2, OFF_DW2TH, BP_TH_DB2, Tt)
                # deferred loads (not needed by the threshold branch): emitted
                # after it so the DMA engine prioritizes the th gate. All on
                # the sync HWDGE queue, ahead of the (blocking) Tt store.
                nc.sync.dma_start(wr[:, :OFF_THW], wp_d[:, :OFF_THW])
                for c in range(2):
                    nc.sync.dma_start(
                        xr[c][:].rearrange("p r c2 -> p (r c2)"),
                        xin_d[c * 128:(c + 1) * 128].rearrange("p r c2 -> p (r c2)"))
                nc.sync.dma_start(qm[:], qm_d[:])
                ba_row = bass.AP(tensor=ba_d.tensor, offset=ba_d.offset,
                                 ap=[[0, 1]] + [list(a) for a in ba_d.ap])
                nc.scalar.dma_start(bar2[:, 0, :], ba_row)
                nc.scalar.dma_start(bar2[:, 1, :], ba_row)
                nc.scalar.dma_start(
                    out_d[1].rearrange("p u c2 -> p (u c2)"),
                    Tt[:].rearrange("p u c2 -> p (u c2)"))

                xr_f = [t[:].rearrange("p r c2 -> p (r c2)") for t in xr]

                # ---- phase 1: e2 (keys), V (values, pos-major) ----
                pe_ctx.close()  # free e1's banks (PSUM pools pop LIFO)
                psc = a2.enter_context(
                    tc.tile_pool(name="psc", bufs=2, space="PSUM"))
                with tc.tile_pool(name="pe2", bufs=2, space="PSUM") as pe2, \
                     tc.tile_pool(name="pv", bufs=2, space="PSUM") as pv:
                    for k0 in range(0, 4096, 512):
                        p = pe2.tile([64, 512], F32, tag="pe", name="pe_t")
                        for c in range(2):
                            o = OFF_WK + c * 64
                            nc.tensor.matmul(p[:], lhsT=wr[:, o:o + 64],
                                             rhs=xr_f[c][:, k0:k0 + 512],
                                             start=(c == 0), stop=(c == 1))
                        nc.scalar.activation(e2r[:, k0:k0 + 512], p[:], AFT.Prelu,
                                             bias=bpk[0:64, BP_BK:BP_BK + 1],
                                             alpha=0.25)
                    for jj in range(NKC // 2):
                        p = pv.tile([128, 2, 256], F32, tag="pv", name="pv_t")
                        for u in range(2):
                            j = 2 * jj + u
                            for c in range(2):
                                o = OFF_WA + c * 256
                                nc.tensor.matmul(p[:, u, :],
                                                 lhsT=xr_f[c][:, j * 128:(j + 1) * 128],
                                                 rhs=wr[:, o:o + 256],
                                                 start=(c == 0), stop=False)
                            nc.tensor.matmul(p[:, u, :], lhsT=onesrV[:],
                                             rhs=bar2[:, u, :], start=False,
                                             stop=True)
                        if jj % 2 == 0:
                            nc.scalar.activation(V[:, 2 * jj:2 * jj + 2, :],
                                                 p[:], AFT.Prelu, alpha=0.25)
                        else:
                            # alternate evictions onto the DVE so the V loop
                            # is not gated by the Act engine (which also runs
                            # the interleaved warm-up exps here)
                            vt = rc.tile([128, 2, 256], F16, tag="vt", name="vt")
                            nc.vector.tensor_scalar_mul(vt[:], p[:], 0.25)
                            nc.vector.tensor_max(V[:, 2 * jj:2 * jj + 2, :],
                                                 vt[:], p[:])
                        # warm the score pipeline: first q-block's QK/exp
                        # pairs interleave with V production, so phase 2
                        # starts with exp'd tiles banked and Act drained
                        if jj in (4, 9, 14):
                            pre[(0, (jj - 4) // 5)] = emit_qk(0, (jj - 4) // 5)

            # ---- phase 2: attention blocks, software-pipelined by one key
            # pair so SV matmuls never wait on the exp eviction; the q-block
            # tail (denominator + normalize) is deferred one extra pair so
            # the PE never waits on the DVE accumulation chain ----
            with tc.tile_pool(name="pxn", bufs=1, space="PSUM") as pxn, \
                 tc.tile_pool(name="prs", bufs=1, space="PSUM") as prs, \
                 tc.tile_pool(name="prb", bufs=1, space="PSUM") as prb:
                state = {}  # per-qblock: xn_ps tiles, racc accumulators

                def get_state(qi):
                    if qi not in state:
                        state[qi] = {
                            "xn": [pxn.tile([128, 512], F32, tag=f"xnp{t}",
                                            name=f"xnps{t}") for t in range(2)],
                            "racc": [rc.tile([128, 512], BF16, tag="racc",
                                             name="racc"),
                                     rc.tile([128, 512], BF16, tag="rac2",
                                             name="rac2")],
                        }
                    return state[qi]

                def emit_sv(qi, jj, E):
                    q0, w = QBLOCKS[qi]
                    st = get_state(qi)
                    for u in range(2):
                        j = 2 * jj + u
                        for t in range(2):
                            nc.tensor.matmul(
                                st["xn"][t][:, :w],
                                lhsT=V[:, j, t * 128:(t + 1) * 128],
                                rhs=E[:, u, :w],
                                start=(jj == 0 and u == 0),
                                stop=(jj == NP - 1 and u == 1))

                def emit_racc(qi, jj, E):
                    q0, w = QBLOCKS[qi]
                    st = get_state(qi)
                    ps = rc.tile([128, 512], BF16, tag="ps", name="ps")
                    nc.vector.tensor_add(ps[:, :w], E[:, 0, :w], E[:, 1, :w])
                    acc = st["racc"][jj % 2]
                    if jj < 2:
                        nc.vector.tensor_copy(acc[:, :w], ps[:, :w])
                    else:
                        nc.vector.tensor_add(acc[:, :w], acc[:, :w], ps[:, :w])

                def emit_tail(qi, Elast):
                    q0, w = QBLOCKS[qi]
                    r0, nr = q0 // 64, w // 64
                    st = state.pop(qi)
                    racc, rac2 = st["racc"]
                    rs = prs.tile([1, 512], F32, tag="rs", name="rs")
                    rb = prb.tile([128, 512], F32, tag="rb", name="rb")
                    # denominator accumulated on the PE across both bf16
                    # accumulators (no DVE combine on the critical path)
                    nc.tensor.matmul(rs[:, :w], lhsT=onesc[:], rhs=racc[:, :w],
                                     start=True, stop=False)
                    nc.tensor.matmul(rs[:, :w], lhsT=onesc[:], rhs=rac2[:, :w],
                                     start=False, stop=True)
                    rrow = rc.tile([1, 512], F32, tag="rrow", name="rrow")
                    nc.vector.reciprocal_approx_fast(rrow[:, :w], rs[:, :w])
                    rrm = rc.tile([1, 512], BF16, tag="rrm", name="rrm")
                    nc.vector.tensor_mul(rrm[:, :w], rrow[:, :w], qm[:, q0:q0 + w])
                    # keep-warm fillers: dep-free matmuls into the rb bank so
                    # the PE never idles (and HAM never re-throttles) while
                    # the reciprocal chain runs on the DVE
                    for _ in range(3):
                        nc.tensor.matmul(rb[:, :w], lhsT=V[:, 0, 0:128],
                                         rhs=Elast[:, 1, :w], start=True, stop=True)
                    nc.tensor.matmul(rb[:, :w], lhsT=onesrB[:], rhs=rrm[:, :w],
                                     start=True, stop=True)
                    rbs = rc.tile([128, 512], F32, tag="rbs", name="rbs")
                    nc.vector.tensor_copy(rbs[:, :w], rb[:, :w])
                    # more fillers into the (now idle) rs bank while the DVE
                    # evicts and normalizes xn
                    for _ in range(4):
                        nc.tensor.matmul(rs[:, :w], lhsT=onesc[:],
                                         rhs=Elast[:, 0, :w], start=True, stop=True)
                    for t in range(2):
                        nc.vector.tensor_mul(
                            xnp[t][:, r0:r0 + nr, 1:65],
                            st["xn"][t][:, :w].rearrange("p (r c2) -> p r c2", c2=64),
                            rbs[:, :w].rearrange("p (r c2) -> p r c2", c2=64))

                # drain the pre-emitted pairs, then run the pipeline
                for jj in (0, 1):
                    emit_sv(0, jj, pre[(0, jj)])
                    emit_racc(0, jj, pre[(0, jj)])
                prev = (0, 2, pre[(0, 2)])
                pending = None
                items = [(0, jj) for jj in range(3, NP)] + \
                        [(qi, jj) for qi in range(1, len(QBLOCKS))
                         for jj in range(NP)]
                for qi, jj in items:
                    E = emit_qk(qi, jj)
                    if pending is not None and jj == 1:
                        emit_tail(*pending)
                        pending = None
                    pqi, pjj, pE = prev
                    emit_sv(pqi, pjj, pE)
                    emit_racc(pqi, pjj, pE)
                    if pjj == NP - 1:
                        pending = (pqi, pE)
                    prev = (qi, jj, E)
                pqi, pjj, pE = prev
                emit_sv(pqi, pjj, pE)
                emit_racc(pqi, pjj, pE)
                emit_tail(pqi, pE)

        # ---- phase 3: binarize branch (att pool closed; SBUF freed) ----
        with tc.tile_pool(name="hc", bufs=3) as hc, \
             tc.tile_pool(name="pcv", bufs=2, space="PSUM") as pcv, \
             tc.tile_pool(name="pct", bufs=2, space="PSUM") as pct, \
             tc.tile_pool(name="ppt", bufs=2, space="PSUM") as ppt:
            _branch_ir(nc, tc, wr, bpk, hc, pcv, pct, ppt, xnp, OFF_BZW,
                       BP_BZ_S1, BP_BZ_B1, OFF_DW1BZ, BP_BZ_S2, BP_BZ_B2,
                       OFF_DW2BZ, BP_BZ_DB2, Tt, out_d=out_d,
                       dmaq=[nc.scalar, nc.sync])

    nc.compile()
    return nc


def _prep(inputs):
    """Host-side parameter prep shared by all cores (numpy, tiny)."""
    g = {k: np.asarray(v, np.float32) for k, v in inputs.items()}
    wpack = np.zeros((128, WCOLS), np.float32)
    wqT = g["wm1"].reshape(64, 256).T
    wpack[:, OFF_WQ:OFF_WQ + 64] = wqT[0:128]
    wpack[:, OFF_WQ + 64:OFF_WQ + 128] = wqT[128:256]
    wkT = g["wm2"].reshape(64, 256).T
    wpack[:, OFF_WK:OFF_WK + 64] = wkT[0:128]
    wpack[:, OFF_WK + 64:OFF_WK + 128] = wkT[128:256]
    waT = g["wa"].reshape(256, 256).T
    wpack[:, OFF_WA:OFF_WA + 256] = waT[0:128]
    wpack[:, OFF_WA + 256:OFF_WA + 512] = waT[128:256]
    for name, off in (("bz_cw", OFF_BZW), ("th_cw", OFF_THW)):
        w3 = g[name].transpose(2, 3, 1, 0).reshape(9, 256, 64)
        for t in range(9):
            for c in range(2):
                wpack[:, off + (t * 2 + c) * 64:off + (t * 2 + c) * 64 + 64] = \
                    w3[t, c * 128:(c + 1) * 128]
    # conv_transpose flips the kernel: tap (di,dj) uses w[1-di, 1-dj];
    # dw1 taps pair-packed into 128-wide outputs
    for name, off in (("bz_dw1", OFF_DW1BZ), ("th_dw1", OFF_DW1TH)):
        d1 = g[name].reshape(4, 64, 64)[::-1]
        for tp in range(2):
            wpack[0:64, off + tp * 128:off + tp * 128 + 64] = d1[2 * tp]
            wpack[0:64, off + tp * 128 + 64:off + tp * 128 + 128] = d1[2 * tp + 1]
    # dw2 block-diagonal [128, 8]: both pair members share the same 64->4 map
    for name, off in (("bz_dw2", OFF_DW2BZ), ("th_dw2", OFF_DW2TH)):
        w2 = g[name].transpose(2, 0, 1, 3).reshape(64, 4)[:, ::-1]
        wpack[0:64, off:off + 4] = w2
        wpack[64:128, off + 4:off + 8] = w2

    bpack = np.zeros((128, BCOLS), np.float32)
    bpack[0:64, BP_BQ] = g["bm1"]
    bpack[0:64, BP_BK] = g["bm2"]
    for pre, (cs1, cb1, cs2, cb2, cdb2) in (
            ("bz", (BP_BZ_S1, BP_BZ_B1, BP_BZ_S2, BP_BZ_B2, BP_BZ_DB2)),
            ("th", (BP_TH_S1, BP_TH_B1, BP_TH_S2, BP_TH_B2, BP_TH_DB2))):
        inv1 = g[f"{pre}_g1"] / np.sqrt(g[f"{pre}_v1"] + EPS)
        bpack[0:64, cs1] = inv1
        bpack[0:64, cb1] = g[f"{pre}_b1"] - g[f"{pre}_m1"] * inv1
        inv2 = g[f"{pre}_g2"] / np.sqrt(g[f"{pre}_v2"] + EPS)
        s2v = np.concatenate([inv2, inv2])
        b2v = np.concatenate([g[f"{pre}_b2"] + (g[f"{pre}_db1"] - g[f"{pre}_m2"]) * inv2] * 2)
        bpack[:, cs2] = s2v
        bpack[:, cb2] = b2v
        bpack[0:8, cdb2] = float(g[f"{pre}_db2"][0])
    return g, wpack, bpack


def kernel(**inputs):
    global LAST_RESULTS
    if "nc" not in _CACHE:
        _CACHE["nc"] = _build()
    nc = _CACHE["nc"]
    g, wpack, bpack = _prep(inputs)
    x = g["x"]  # [4, 256, 64, 64]
    wpack16 = wpack.astype(np.float16)
    ba16 = g["ba"].astype(np.float16)

    in_maps = []
    for core in range(8):
        b, s = core % 4, core // 4
        xpad = np.zeros((256, 34, 66), np.float32)
        qmask = np.ones((1, NQ), np.float32)
        if s == 0:
            xpad[:, 1:34, 1:65] = x[b][:, 0:33]
            qmask[0, 0:64] = 0.0
        else:
            xpad[:, 0:33, 1:65] = x[b][:, 31:64]
            qmask[0, 33 * 64:] = 0.0
        in_maps.append({"xin": np.ascontiguousarray(x[b]).astype(np.float16),
                        "xpad": xpad.astype(np.float16),
                        "qmask": qmask, "wpack": wpack16, "bpack": bpack,
                        "ba": ba16})

    br = run_bass_kernel_spmd(
        nc, in_maps, core_ids=list(range(8)),
        trace=os.environ.get("KERNEL_TRACE", "0") == "1")
    LAST_RESULTS = br

    out = np.zeros((4, 3, 256, 256), np.float32)
    for core in range(8):
        b, s = core % 4, core // 4
        # raw [ch, m*4 + (di2*2+dj2), tp, r*64+c]:
        # final row = 4r + 2*tp + di2, col = 4c + 2*m + dj2
        raw = br.results[core]["out"].reshape(3, 2, 2, 2, 2, 32, 64)
        half = raw.transpose(0, 5, 4, 2, 6, 1, 3).reshape(3, 128, 256)
        out[b, :, 128 * s:128 * (s + 1), :] = half
    return out
